# revision 17
# baseline (speedup 1.0000x reference)
"""Biaffine span head kernel for Trainium2 (Bass/Tile), SPMD over 8 NeuronCores.

Math (per batch element b):
    Hu   = H @ U                        [L, d]
    a    = H @ w1                       [L]
    c    = H @ w2                       [L]
    s[l, off] = <Hu[l], H[l+off]> + a[l] + b + c[l+off]      off in [0, 30)
    out[l, off] = s if mask[l]*mask_pad[l+off] != 0 else -1e9

Sharding: data-parallel over batch B=8 across the 8 cores (one batch row per
core); everything is local per shard, no collectives.

Per-core schedule: software pipeline over 4 l-blocks of 512. All matmul
operands are fp16 (fp32 accumulation in PSUM). Per block: H tiles DMA in,
cast to fp16, transpose on the PE, then HuT = (H@U)^T accumulates over 8
K-chunks, then the banded Gram blocks [128,158] run.

The affine terms AND the mask fixup are folded into the band matmul as a
4-row extra contraction chunk:
    r0: lhsT 1s        x rhs c[l']          -> +c
    r1: lhsT a[l]+b    x rhs 1s             -> +a+b
    r2: lhsT S1(m-1)   x rhs S2*m[l']       -> +S1*S2*(m[l]-1)*m[l']
    r3: lhsT S2        x rhs S1*(m[l']-1)   -> +S1*S2*(m[l']-1)
with S1=32768, S2=30512 (both fp16-exact, S1*S2 = 999_817_216 ~ 1e9), so
masked entries come out ~ -1e9 (rel err vs the reference's exact -1e9 is
~2e-4 against the 1e9 denominator). The band matmul therefore produces the
FINAL output values; the old fixup phase reduces to a single DRAM->DRAM
diagonal-gather DMA per block (band scratch row stride NB+1 = diagonal is
affine in DRAM address space), fully overlapped with the next block's
compute.
"""

import os
import sys

import numpy as np

for _p in ("/opt/trn_rl_repo",):
    if _p not in sys.path and os.path.isdir(_p):
        sys.path.insert(0, _p)

B = 8
L = 2048
D = 1024
K = 30          # band width (MAX_ANSWER_LEN)
P = 128         # partitions
NB = P + K      # 158: band matmul window
KC = D // P     # 8 contraction chunks
NBLK = 512      # l-block width
LBN = L // NBLK  # 4
TPB = NBLK // P  # 4 l-tiles per block
LT = L // P     # 16 l-tiles
LPAD = L + 32   # padded H^T / row-vector width (2080)
S1 = 32768.0    # fp16-exact mask scales; S1*S2 = 999_817_216 ~ 1e9
S2 = 30512.0

_CACHE = {}


def _build_nc():
    import contextlib

    import concourse.bass as bass
    import concourse.tile as tile
    from concourse import bacc, mybir

    f32 = mybir.dt.float32
    f16 = mybir.dt.float16
    i32 = mybir.dt.int32

    nc = bacc.Bacc("TRN2", target_bir_lowering=False, debug=False, num_devices=B)

    H_h = nc.dram_tensor("H", [L, D], f32, kind="ExternalInput")
    mask_h = nc.dram_tensor("mask", [L], i32, kind="ExternalInput")
    U_h = nc.dram_tensor("U", [D, D], f32, kind="ExternalInput")
    w_h = nc.dram_tensor("w", [2 * D], f32, kind="ExternalInput")
    b_h = nc.dram_tensor("b", [1], f32, kind="ExternalInput")
    out_h = nc.dram_tensor("out", [L, K], f32, kind="ExternalOutput")

    band_h = nc.dram_tensor("band_scratch", [LT, P, NB], f32)

    ident_h = nc.inline_tensor(np.eye(P, dtype=np.float16), name="ident_const")
    # constant rows for the extra-contraction chunk: [1s, S2, 0s, -S1]
    crows = np.zeros((4, LPAD), dtype=np.float16)
    crows[0, :] = 1.0
    crows[1, :] = S2
    crows[3, :] = -S1
    crows_h = nc.inline_tensor(crows, name="crows_const")

    H = H_h.ap()

    def dap(h, off, dims):
        # DRAM access pattern helper: dims = [(stride_elems, count), ...]
        return bass.AP(h, off, [list(d) for d in dims])

    with tile.TileContext(nc) as tc, tc.tile_pool(name="perm", bufs=1) as perm_pool:

        def perm(shape, dtype, name):
            return perm_pool.tile(shape, dtype, name=name, tag=name)

        # ---- persistent SBUF tensors (fp16 datapath) ----
        ident_sb = perm([P, P], f16, name="ident_sb")
        nc.sync.dma_start(ident_sb[:], ident_h.ap())

        U16 = [perm([P, KC, P], f16, name=f"U16_{dc}") for dc in range(KC)]
        HT = [perm([P, LPAD], f16, name=f"HT{kk}") for kk in range(KC)]
        HuT = [perm([P, L], f16, name=f"HuT{kk}") for kk in range(KC)]
        w16 = perm([P, KC, 2], f16, name="w16")
        b_sb = perm([1, 1], f32, name="b_sb")
        m32 = perm([1, L], f32, name="m32")
        # partition-0 staging rows (engine writes must start at partition 0)
        trowA = perm([1, L], f16, name="trowA")
        trowB = perm([1, L], f16, name="trowB")
        # extra-contraction row tiles for the band matmul (see module docstring)
        lx = perm([4, L], f16, name="lx")
        rx = perm([4, LPAD], f16, name="rx")

        for kk in range(KC):
            nc.gpsimd.memset(HT[kk][:, L:LPAD], 0.0)
        cr = crows_h.ap()
        nc.sync.dma_start(lx[0:1, :], cr[0:1, 0:L])        # r0 lhsT: ones
        nc.sync.dma_start(lx[3:4, :], cr[1:2, 0:L])        # r3 lhsT: S2 const
        nc.sync.dma_start(rx[1:2, :], cr[0:1, :])          # r1 rhs: ones
        nc.sync.dma_start(rx[0:1, L:LPAD], cr[2:3, 0:32])  # c pad: 0
        nc.sync.dma_start(rx[2:3, L:LPAD], cr[2:3, 0:32])  # S2*m pad: 0
        nc.sync.dma_start(rx[3:4, L:LPAD], cr[3:4, 0:32])  # S1*(m-1) pad: -S1
        nc.sync.dma_start(b_sb[:], dap(b_h, 0, [(1, 1), (1, 1)]))

        with contextlib.ExitStack() as ctx:
            hstage_pool = ctx.enter_context(tc.tile_pool(name="hstage", bufs=6))
            h16_pool = ctx.enter_context(tc.tile_pool(name="h16", bufs=8))
            wstage_pool = ctx.enter_context(tc.tile_pool(name="wstage", bufs=3))
            trps = ctx.enter_context(tc.tile_pool(name="trpsum", bufs=2, space="PSUM"))
            hups = ctx.enter_context(tc.tile_pool(name="hupsum", bufs=2, space="PSUM"))
            bps = ctx.enter_context(tc.tile_pool(name="bandpsum", bufs=3, space="PSUM"))
            acps = ctx.enter_context(tc.tile_pool(name="acpsum", bufs=1, space="PSUM"))
            bsb_pool = ctx.enter_context(tc.tile_pool(name="bandsb", bufs=2))
            mrow_pool = ctx.enter_context(tc.tile_pool(name="mrow", bufs=3))

            hstages = {}

            def load_block(lb):
                for i in range(TPB):
                    l0 = (lb * TPB + i) * P
                    hs = hstage_pool.tile([P, D], f32, name="hs", tag="hs")
                    nc.sync.dma_start(hs[:], H[l0:l0 + P, :])
                    h16 = h16_pool.tile([P, D], f16, name="h16t", tag="h16t")
                    nc.vector.tensor_copy(h16[:], hs[:])
                    hstages[(lb, i)] = h16

            def transposes(lb):
                j0 = lb * NBLK
                for kk in range(KC):
                    tp = trps.tile([P, NBLK], f16, name="tp", tag="tp")
                    for i in range(TPB):
                        nc.tensor.matmul(
                            tp[:, i * P:(i + 1) * P],
                            lhsT=hstages[(lb, i)][:, kk * P:(kk + 1) * P],
                            rhs=ident_sb[:],
                            is_transpose=True,
                            start=(i == 0),
                            stop=(i == TPB - 1),
                        )
                    nc.scalar.copy(HT[kk][:, j0:j0 + NBLK], tp[:])
                for i in range(TPB):
                    del hstages[(lb, i)]

            def setup_weights():
                # emitted after block-0 H loads so the H DMAs go out first;
                # small tensors (w, mask) first, then U in dc-chunks so
                # gemm1(0) can start as soon as chunk 0 lands.
                # w column order SWAPPED (w2 first): acp row0 = c, row1 = a,
                # so each lands on the partition its lx/rx row needs
                # (engines cannot shift partitions).
                w_s = wstage_pool.tile([P, KC, 2], f32, name="w_s", tag="w_s")
                nc.scalar.dma_start(w_s[:, :, 0], dap(w_h, D, [(1, P), (P, KC)]))
                nc.scalar.dma_start(w_s[:, :, 1], dap(w_h, 0, [(1, P), (P, KC)]))
                nc.scalar.copy(w16[:], w_s[:])
                # mask rows: engines must access partition ranges starting
                # at partition 0 (mod 32), so compute into partition-0
                # staging rows and DMA (no partition constraint) into place.
                m_i = mrow_pool.tile([1, L], i32, name="m_i")
                nc.scalar.dma_start(m_i[:], dap(mask_h, 0, [(L, 1), (1, L)]))
                nc.vector.tensor_copy(m32[:], m_i[:])
                # S1*(m-1)  (0 where m=1, -S1 where m=0)
                nc.vector.tensor_scalar(
                    trowA[:], in0=m32[:], scalar1=S1, scalar2=-S1,
                    op0=mybir.AluOpType.mult, op1=mybir.AluOpType.add,
                )
                # S2*m
                nc.vector.tensor_scalar_mul(trowB[:], m32[:], S2)
                nc.sync.dma_start(lx[2:3, :], trowA[:])    # r2 lhsT
                nc.sync.dma_start(rx[3:4, 0:L], trowA[:])  # r3 rhs
                nc.sync.dma_start(rx[2:3, 0:L], trowB[:])  # r2 rhs
                for dc in range(KC):
                    u_s = wstage_pool.tile([P, KC, P], f32, name="u_s", tag="u_s")
                    nc.scalar.dma_start(
                        u_s[:],
                        dap(U_h, dc * P, [(D, P), (P * D, KC), (1, P)]),
                    )
                    nc.scalar.copy(U16[dc][:], u_s[:])

            def gemm1(lb):
                j0 = lb * NBLK
                for dc in range(KC):
                    hp = hups.tile([P, NBLK], f32, name="hp", tag="hp")
                    for kk in range(KC):
                        nc.tensor.matmul(
                            hp[:],
                            lhsT=U16[dc][:, kk, :],
                            rhs=HT[kk][:, j0:j0 + NBLK],
                            start=(kk == 0),
                            stop=(kk == KC - 1),
                        )
                    nc.vector.tensor_copy(HuT[dc][:, j0:j0 + NBLK], hp[:])

            def ac_gemm(lb):
                # c (row 0) and a (row 1) for block lb; feeds the lx/rx rows
                # consumed by band(lb-1) (window spills 30 cols into block lb)
                # and band(lb). b is folded into the c row (term 1*(c+b)).
                j0 = lb * NBLK
                acp = acps.tile([2, NBLK], f32, name="acp", tag="acp")
                for kk in range(KC):
                    nc.tensor.matmul(
                        acp[:],
                        lhsT=w16[:, kk, :],
                        rhs=HT[kk][:, j0:j0 + NBLK],
                        start=(kk == 0),
                        stop=(kk == KC - 1),
                    )
                # r0 rhs: c + b (acp row0 = c, partition 0 -> direct)
                nc.vector.tensor_scalar_add(
                    rx[0:1, j0:j0 + NBLK], acp[0:1, :], b_sb[0:1, 0:1]
                )
                # r1 lhsT: a (acp row1; engines can't start at partition 1,
                # so cast both rows into a partition-0 staging tile and DMA
                # row 1 into place)
                t16 = mrow_pool.tile([2, NBLK], f16, name="t16", tag="t16")
                nc.vector.tensor_copy(t16[:], acp[:])
                nc.sync.dma_start(lx[1:2, j0:j0 + NBLK], t16[1:2, :])

            def band(lb):
                bsb = bsb_pool.tile([P, TPB, NB], f32, name="bsb", tag="bsb")
                for i in range(TPB):
                    l0 = (lb * TPB + i) * P
                    bp = bps.tile([P, NB], f32, name="bp", tag="bp")
                    for kk in range(KC):
                        nc.tensor.matmul(
                            bp[:],
                            lhsT=HuT[kk][:, l0:l0 + P],
                            rhs=HT[kk][:, l0:l0 + NB],
                            start=(kk == 0),
                            stop=False,
                        )
                    nc.tensor.matmul(
                        bp[:],
                        lhsT=lx[:, l0:l0 + P],
                        rhs=rx[:, l0:l0 + NB],
                        start=False,
                        stop=True,
                    )
                    nc.vector.tensor_copy(bsb[:, i, :], bp[:])
                nc.scalar.dma_start(
                    dap(band_h, lb * TPB * P * NB,
                        [(NB, P), (P * NB, TPB), (1, NB)]),
                    bsb[:],
                )
                # final values: gather the 30-wide diagonal band straight to
                # the output (DRAM->DRAM; diagonal = row stride NB+1)
                nc.sync.dma_start(
                    dap(out_h, lb * TPB * P * K, [(K, P), (P * K, TPB), (1, K)]),
                    dap(band_h, lb * TPB * P * NB,
                        [(NB + 1, P), (P * NB, TPB), (1, K)]),
                )

            # ---- pipeline ----
            # PE order per iteration: gemm1(lb) runs BEFORE transposes(lb+1)
            # so the PE isn't stalled waiting on block lb+1's loads/casts.
            load_block(0)
            setup_weights()
            transposes(0)
            ac_gemm(0)
            for lb in range(LBN):
                if lb + 1 < LBN:
                    load_block(lb + 1)
                gemm1(lb)
                if lb + 1 < LBN:
                    transposes(lb + 1)
                    ac_gemm(lb + 1)
                band(lb)

    nc.compile()
    return nc


def get_nc():
    if "nc" not in _CACHE:
        _CACHE["nc"] = _build_nc()
    return _CACHE["nc"]


def kernel(H, attention_mask, U, w, b):
    from concourse.bass_utils import run_bass_kernel_spmd

    nc = get_nc()
    H = np.asarray(H, dtype=np.float32)
    attention_mask = np.asarray(attention_mask, dtype=np.int32)
    U_np = np.ascontiguousarray(np.asarray(U, dtype=np.float32))
    w_np = np.ascontiguousarray(np.asarray(w, dtype=np.float32).reshape(-1))
    b_np = np.ascontiguousarray(np.asarray(b, dtype=np.float32).reshape(-1))

    in_maps = []
    for i in range(B):
        in_maps.append({
            "H": np.ascontiguousarray(H[i]),
            "mask": np.ascontiguousarray(attention_mask[i]),
            "U": U_np,
            "w": w_np,
            "b": b_np,
        })
    res = run_bass_kernel_spmd(nc, in_maps, list(range(B)))
    return np.stack([res.results[i]["out"] for i in range(B)], axis=0)
